# revision 3
# baseline (speedup 1.0000x reference)
"""Multi-head attention for 8 Trainium2 NeuronCores.

Sharding: 4 batches x 2 head-groups. Core c handles batch c%4 and heads
(c//4)*8 .. +8 (four head-pairs of 128 dims). Each core writes a full-size
[E, S] fp32 partial of its batch's output; the host sums the two head-group
partials per batch and adds the output bias.

Per-core pipeline (pairs p=0..3):
  proj(p): Q/K/V^T [128, S] bf16 via PE (Wq, bq pre-scaled by 1/8 host-side
           so QK directly yields energy/sqrt(D)). K is written into two
           zero-padded per-head tiles so QK runs as full K=128 matmuls
           (K=64 matmuls measured ~2x slower per streamed column on HW,
           with or without tile_position row-packing).
  vt(p):   128x128 PE transposes of V^T -> v1 [128k, 16ck, 2h, 128]
           (cols 0-63 hold SIXTY-FOUR ones-columns, cols 64-127 the head
           dims, so the AV matmul itself writes the softmax denominator
           REPLICATED across psum partitions 0-63 - a free partition
           broadcast; streaming cost is N-bound so M=128 vs 65 is free).
  attn(p,cq): per key-chunk ck: 2 QK matmuls -> psum [128, 2h, 512] ->
           1024-wide Exp -> etp bf16; AV accumulates po [128, 2h, 512] fp32
           over 16 ck.  Norm: one DVE copy evacuates the dims half
           (po[64:128] -> base-0 tile; psum freed), one base-0 DVE
           reciprocal_approx_fast reads the replicated dens straight from
           psum partitions 0-63, two muls -> ot bf16.  No ACT and no gpsimd
           on the normalization path (ACT stays a pure exp stream; gpsimd
           partition_broadcast and custom-DVE ops read absolute partition 0,
           and two-input DVE ops need MATCHED input base partitions - this
           layout keeps every op's APs legal).
  fc:      yt partial, contraction over all 4 pairs; fc(co, sc) units become
           PE filler inside pair-3's attention as each sc's norms complete.
Filler units are interleaved between attention units in PE program order,
spread EVENLY across each pair-phase's blocks (units queued before a block
outrank its qk/av in scheduler priority, so front-loading them starves the
ACT-paced block tail).  During the xt-DMA-gated rampup, the first 8 proj
units borrow the idle qk/av psum banks so ~13us of PE work overlaps the
input DMA.  PSUM: qk 2x2 + av 2 + mm 2x1 = 8 banks.
"""

from contextlib import ExitStack, contextmanager

import numpy as np
import ml_dtypes

import concourse.bacc as bacc
import concourse.mybir as mybir
import concourse.tile as tile
from concourse.bass import ts
from concourse.masks import make_identity
from concourse.bass_utils import run_bass_kernel_spmd
from concourse.bass_interp import get_hw_module


@contextmanager
def _pinned_act_tables():
    """Pin every ACT function to the one table set containing both exp and
    ln so the table-load pass hoists a single LoadActFuncSet instead of
    reloading (~2.7us on HW). Scoped to compile."""
    orig = bacc.get_activation_tables

    def pinned(arch):
        return {
            name: (fns if name == "natural_log_exp_and_others" else set())
            for name, fns in orig(arch).items()
        }

    bacc.get_activation_tables = pinned
    try:
        yield
    finally:
        bacc.get_activation_tables = orig


FP32 = mybir.dt.float32
BF16 = mybir.dt.bfloat16

E = 1024
S = 2048
NB = 4
NG = 2
GD = E // NG    # dims per head group = 512
NP = GD // 128  # head-pairs per core = 4
D = 64
NJ = E // 128
SQ = 512
NQ = S // SQ
NK = S // 128
NO = E // 128
N_CORES = 8
ETP_BUFS = 6


def _build():
    with _pinned_act_tables():
        return _build_inner()


def _build_inner():
    nc = bacc.Bacc("TRN2", target_bir_lowering=False, debug=True)

    xt = nc.dram_tensor("xt", [E, S], BF16, kind="ExternalInput")
    wq = nc.dram_tensor("wq", [128, NJ, GD], BF16, kind="ExternalInput")
    wk = nc.dram_tensor("wk", [128, NJ, GD], BF16, kind="ExternalInput")
    wv = nc.dram_tensor("wv", [128, NJ, GD], BF16, kind="ExternalInput")
    wot = nc.dram_tensor("wot", [128, NP, E], BF16, kind="ExternalInput")
    bq = nc.dram_tensor("bq", [128, NP], FP32, kind="ExternalInput")
    bk = nc.dram_tensor("bk", [128, NP], FP32, kind="ExternalInput")
    bv = nc.dram_tensor("bv", [128, NP], FP32, kind="ExternalInput")
    yt = nc.dram_tensor("yt", [E, S], FP32, kind="ExternalOutput")

    with tile.TileContext(nc) as tc, ExitStack() as ctx:
        const = ctx.enter_context(tc.tile_pool(name="const", bufs=1))
        wq_sb = const.tile([128, NJ, GD], BF16)
        wk_sb = const.tile([128, NJ, GD], BF16)
        wv_sb = const.tile([128, NJ, GD], BF16)
        wot_sb = const.tile([128, NP, E], BF16)
        bq_sb = const.tile([128, NP], FP32)
        bk_sb = const.tile([128, NP], FP32)
        bv_sb = const.tile([128, NP], FP32)
        ident = const.tile([128, 128], BF16)
        ktz = []
        for p in range(NP):
            k0 = const.tile([128, S], BF16, name=f"ktz0_{p}")
            k1 = const.tile([128, S], BF16, name=f"ktz1_{p}")
            nc.vector.memset(k0[D:128, :], 0.0)
            nc.vector.memset(k1[0:D, :], 0.0)
            ktz.append((k0, k1))
        v1c = []
        for p in range(NP):
            vc = const.tile([128, NK, 2, 128], BF16, name=f"v1c_{p}")
            nc.vector.memset(vc[:, :, :, 0:D], 1.0)
            v1c.append(vc)
        nc.sync.dma_start(wq_sb[:], wq[:, :, :])
        nc.sync.dma_start(wk_sb[:], wk[:, :, :])
        nc.sync.dma_start(wv_sb[:], wv[:, :, :])
        nc.sync.dma_start(wot_sb[:], wot[:, :, :])
        nc.sync.dma_start(bq_sb[:], bq[:, :])
        nc.sync.dma_start(bk_sb[:], bk[:, :])
        nc.sync.dma_start(bv_sb[:], bv[:, :])
        make_identity(nc, ident[:, :])

        xt_pool = ctx.enter_context(tc.tile_pool(name="xt_pool", bufs=1))
        qkv_pool = ctx.enter_context(tc.tile_pool(name="qkv_pool", bufs=2))
        et_pool = ctx.enter_context(tc.tile_pool(name="et_pool", bufs=ETP_BUFS))
        ot_pool = ctx.enter_context(tc.tile_pool(name="ot_pool", bufs=2))
        st_pool = ctx.enter_context(tc.tile_pool(name="st_pool", bufs=2))
        nrm_pool = ctx.enter_context(tc.tile_pool(name="nrm_pool", bufs=2))
        rb_pool = ctx.enter_context(tc.tile_pool(name="rb_pool", bufs=1))
        pp_qk = ctx.enter_context(tc.tile_pool(name="pp_qk", bufs=2, space="PSUM"))
        pp_av = ctx.enter_context(tc.tile_pool(name="pp_av", bufs=1, space="PSUM"))
        pp_mm = ctx.enter_context(tc.tile_pool(name="pp_mm", bufs=2, space="PSUM"))

        xt_tiles = []
        for j in range(NJ):
            xj = xt_pool.tile([128, S], BF16, name=f"xt_sb{j}", tag=f"xt{j}")
            nc.sync.dma_start(xj[:, :], xt[j * 128 : (j + 1) * 128, :])
            xt_tiles.append(xj)

        qt = [None] * NP
        vt = [None] * NP
        v1 = [None] * NP
        ot = [None] * NP

        def proj_unit(kind, p, sc, borrow=None):
            w_sb, b_sb, dstl = {
                "q": (wq_sb, bq_sb, qt),
                "k": (wk_sb, bk_sb, None),
                "v": (wv_sb, bv_sb, vt),
            }[kind]
            if dstl is not None and dstl[p] is None:
                dstl[p] = qkv_pool.tile(
                    [128, S], BF16, name=f"{kind}t{p % 2}", tag=f"{kind}t{p % 2}"
                )
            if borrow is not None:
                ps = borrow
            else:
                ps = pp_mm.tile([128, SQ], FP32, name="ps_proj", tag="mm")
            for j in range(NJ):
                nc.tensor.matmul(
                    ps[:, :],
                    w_sb[:, j, ts(p, 128)],
                    xt_tiles[j][:, ts(sc, SQ)],
                    start=(j == 0),
                    stop=(j == NJ - 1),
                )
            if kind == "k":
                nc.vector.tensor_scalar_add(
                    ktz[p][0][0:D, ts(sc, SQ)], ps[0:D, :], b_sb[0:D, p : p + 1]
                )
                nc.vector.tensor_scalar_add(
                    ktz[p][1][D:128, ts(sc, SQ)], ps[D:128, :], b_sb[D:128, p : p + 1]
                )
            else:
                nc.vector.tensor_scalar_add(
                    dstl[p][:, ts(sc, SQ)], ps[:, :], b_sb[:, p : p + 1]
                )

        def vt_unit(p, cks):
            if v1[p] is None:
                v1[p] = v1c[p]
            for ck in range(cks * 4, cks * 4 + 4):
                pt = pp_mm.tile([128, 128], BF16, name="pt_vt", tag="mm")
                nc.tensor.transpose(pt[:, :], vt[p][:, ts(ck, 128)], ident[:, :])
                nc.vector.tensor_copy(
                    v1[p][:, ck, :, D:128],
                    pt[:, :].rearrange("p (h d) -> p h d", h=2),
                )

        def fc_unit(co, sc):
            stw = st_pool.tile([128, SQ], FP32, name="st_fc", tag="st")
            pf = pp_mm.tile([128, SQ], FP32, name="pf_fc", tag="mm")
            for p in range(NP):
                nc.tensor.matmul(
                    pf[:, :],
                    wot_sb[:, p, ts(co, 128)],
                    ot[p][:, ts(sc, SQ)],
                    start=(p == 0),
                    stop=(p == NP - 1),
                )
            nc.vector.tensor_copy(stw[:, :], pf[:, :])
            nc.sync.dma_start(yt[co * 128 : (co + 1) * 128, ts(sc, SQ)], stw[:, :])

        queue = []

        def pump(n):
            k = 0
            while queue and k < n:
                queue.pop(0)()
                k += 1

        def attn_block(p, cq):
            etps = [None] * NK

            def qk_unit(ck):
                psw = pp_qk.tile([128, 2, SQ], FP32, name="psw", tag="qk")
                for h in range(2):
                    nc.tensor.matmul(
                        psw[:, h, :],
                        ktz[p][h][:, ts(ck, 128)],
                        qt[p][:, ts(cq, SQ)],
                        start=True,
                        stop=True,
                    )
                etp = et_pool.tile([128, 2, SQ], BF16, name="etp", tag="et")
                nc.scalar.activation(
                    etp[:, :, :], psw[:, :, :], mybir.ActivationFunctionType.Exp
                )
                etps[ck] = etp

            def av_unit(ck, po):
                for h in range(2):
                    nc.tensor.matmul(
                        po[:, h, :],
                        v1[p][:, ck, h, :],
                        etps[ck][:, h, :],
                        start=(ck == 0),
                        stop=(ck == NK - 1),
                    )

            # trace-time guard: av_unit emission requires v1[p] (vt units
            # sit in the queue during the startup block)
            while queue and v1[p] is None:
                pump(1)
            # Spread the queued filler units evenly across the remaining
            # blocks of this pair-phase: units queued before this block have
            # EARLIER scheduler priority than the block's qk/av, so pumping
            # them all early front-loads the PE and starves the (ACT-paced)
            # block tail.
            blocks_left = NQ - cq
            if p == 0:
                pops = len(queue)
            elif p == NP - 1:
                pops = min(len(queue), NO)
            else:
                pops = min(len(queue), -(-len(queue) // blocks_left))
            space = NK / pops if pops else NK + 1
            po = pp_av.tile([128, 2, SQ], FP32, name="po", tag="av")
            popped = 0
            for ck in range(NK):
                if ck >= 3:
                    av_unit(ck - 3, po)
                qk_unit(ck)
                want = min(pops, int((ck + 1) / space))
                if want > popped:
                    pump(want - popped)
                    popped = want
            for ck in range(NK - 3, NK):
                av_unit(ck, po)
                if popped < pops:
                    pump(1)
                    popped += 1

            if ot[p] is None:
                ot[p] = ot_pool.tile([128, S], BF16, name=f"ot{p % 2}", tag=f"ot{p % 2}")
            # po partitions 0-63 hold den replicated (v1 ones-cols), 64-127
            # the head dims.  Custom-DVE ops require ALL APs at base
            # partition 0, and two-input DVE ops require the input bases to
            # MATCH - hence: recip reads den straight from psum at base 0;
            # dims evacuate hi->lo (legal) so the muls see matched base-0
            # inputs.
            poc = nrm_pool.tile([D, 2, SQ], FP32, name="poc")
            nc.vector.tensor_copy(poc[:, :, :], po[D:128, :, :])
            rb = rb_pool.tile([D, 2, SQ], FP32, name="rb")
            nc.vector.reciprocal_approx_fast(rb[:, :, :], po[0:D, :, :])
            nc.vector.tensor_mul(ot[p][0:D, ts(cq, SQ)], poc[:, 0, :], rb[:, 0, :])
            nc.vector.tensor_mul(ot[p][D:128, ts(cq, SQ)], poc[:, 1, :], rb[:, 1, :])

        proj_unit("q", 0, 0)
        proj_unit("k", 0, 0)
        rq1 = pp_qk.tile([128, 2, SQ], FP32, name="ramp_qk1", tag="qk")
        rq2 = pp_qk.tile([128, 2, SQ], FP32, name="ramp_qk2", tag="qk")
        rav = pp_av.tile([128, 2, SQ], FP32, name="ramp_av", tag="av")
        proj_unit("k", 0, 1, borrow=rq1[:, 0, :])
        proj_unit("k", 0, 2, borrow=rq1[:, 1, :])
        proj_unit("k", 0, 3, borrow=rq2[:, 0, :])
        proj_unit("v", 0, 0, borrow=rq2[:, 1, :])
        proj_unit("v", 0, 1, borrow=rav[:, 0, :])
        proj_unit("v", 0, 2, borrow=rav[:, 1, :])
        queue.append(lambda: proj_unit("v", 0, 3))
        for cks in range(4):
            queue.append(lambda cks=cks: vt_unit(0, cks))
        for sc in range(1, NQ):
            queue.append(lambda sc=sc: proj_unit("q", 0, sc))

        for p in range(NP):
            if p + 1 < NP:
                for kind in ("k", "q", "v"):
                    for sc in range(NQ):
                        queue.append(
                            lambda k=kind, sc=sc, p2=p + 1: proj_unit(k, p2, sc)
                        )
                for cks in range(4):
                    queue.append(lambda cks=cks, p2=p + 1: vt_unit(p2, cks))
            for cq in range(NQ):
                attn_block(p, cq)
                if p == NP - 1 and cq < NQ - 1:
                    for co in range(NO):
                        queue.append(lambda co=co, sc=cq: fc_unit(co, sc))
        pump(len(queue))
        # Final-slice fc: the 8 fc(co, sc=3) units would otherwise serialize
        # after the last norm on 2 mm banks.  Pre-run their p0-2 partial
        # matmuls on the qk/av banks (free once the last block's exps/norm
        # consume psw/po) during the last block's norm wait; only the p3
        # matmul + store remain in the tail.
        sc = NQ - 1
        fps = []
        fps.append(pp_mm.tile([128, SQ], FP32, name="fcs_mm0", tag="mm"))
        fps.append(pp_mm.tile([128, SQ], FP32, name="fcs_mm1", tag="mm"))
        fq1 = pp_qk.tile([128, 2, SQ], FP32, name="fcs_qk1", tag="qk")
        fq2 = pp_qk.tile([128, 2, SQ], FP32, name="fcs_qk2", tag="qk")
        fav = pp_av.tile([128, 2, SQ], FP32, name="fcs_av", tag="av")
        fps += [fq1[:, 0, :], fq1[:, 1, :], fq2[:, 0, :], fq2[:, 1, :],
                fav[:, 0, :], fav[:, 1, :]]
        for co in range(NO):
            for p2 in range(NP - 1):
                nc.tensor.matmul(
                    fps[co][:, :],
                    wot_sb[:, p2, ts(co, 128)],
                    ot[p2][:, ts(sc, SQ)],
                    start=(p2 == 0),
                    stop=False,
                )
        for co in range(NO):
            nc.tensor.matmul(
                fps[co][:, :],
                wot_sb[:, NP - 1, ts(co, 128)],
                ot[NP - 1][:, ts(sc, SQ)],
                start=False,
                stop=True,
            )
            stw = st_pool.tile([128, SQ], FP32, name="st_fcs", tag="st")
            nc.vector.tensor_copy(stw[:, :], fps[co][:, :])
            nc.sync.dma_start(
                yt[co * 128 : (co + 1) * 128, ts(sc, SQ)], stw[:, :]
            )

    nc.compile()
    nc.m = get_hw_module(nc.m)
    return nc


_NC_CACHE = None


def _get_nc():
    global _NC_CACHE
    if _NC_CACHE is None:
        _NC_CACHE = _build()
    return _NC_CACHE


def _bf(a):
    return np.ascontiguousarray(a).astype(ml_dtypes.bfloat16)


def make_in_maps(x, Wq, bq, Wk, bk, Wv, bv, Wo):
    x = np.asarray(x, dtype=np.float32)
    in_maps = []
    for c in range(N_CORES):
        b, hg = c % NB, c // NB
        sl = slice(hg * GD, (hg + 1) * GD)

        def wslice(W, scale=1.0):
            wt = (np.asarray(W, dtype=np.float32)[sl] * scale).T  # [E, GD]
            return _bf(wt.reshape(NJ, 128, GD).transpose(1, 0, 2))

        in_maps.append(
            {
                "xt": _bf(x[b].T),
                "wq": wslice(Wq, 0.125),
                "wk": wslice(Wk),
                "wv": wslice(Wv),
                "wot": _bf(
                    np.ascontiguousarray(np.asarray(Wo, dtype=np.float32)[:, sl].T)
                    .reshape(NP, 128, E)
                    .transpose(1, 0, 2)
                ),
                "bq": (np.asarray(bq, dtype=np.float32)[sl] * 0.125)
                .reshape(NP, 128)
                .T.copy(),
                "bk": np.asarray(bk, dtype=np.float32)[sl].reshape(NP, 128).T.copy(),
                "bv": np.asarray(bv, dtype=np.float32)[sl].reshape(NP, 128).T.copy(),
            }
        )
    return in_maps


def unshard(res, bo):
    bo = np.asarray(bo, dtype=np.float32)
    outs = []
    for b in range(NB):
        ytp = res[b]["yt"].astype(np.float32) + res[NB + b]["yt"].astype(np.float32)
        outs.append(ytp.T + bo)
    return np.ascontiguousarray(np.stack(outs), dtype=np.float32)


def kernel(x, Wq, bq, Wk, bk, Wv, bv, Wo, bo):
    nc = _get_nc()
    in_maps = make_in_maps(x, Wq, bq, Wk, bk, Wv, bv, Wo)
    res = run_bass_kernel_spmd(nc, in_maps, list(range(N_CORES)))
    return unshard(res.results, bo)
